# revision 40
# baseline (speedup 1.0000x reference)
"""DeeperSet aggregation kernel for 8 Trainium2 NeuronCores.

Strategy: data-parallel over contiguous graph-id ranges (2048 graphs/core).
Segment boundaries are host-known (batch is an input), so segment-sum and
the xg[batch] gather are expressed as matmuls against host-built one-hot
tiles.  LayerNorm (gamma=1, beta=0, biases=0 in this model) reduces to a
per-node positive scale r = 1/sqrt(mean(u^2)+eps) with mean-centering folded
into the weights on the host.  r commutes through ReLU and the segment-sum,
so it is applied to the (half-width) one-hot rows instead of the
activations.  Elementwise work is batched over GRP-tile super-groups to
amortize per-instruction overheads, and emission is software-pipelined
(one stats-batch and one segment-phase of lookahead) so the PE never
stalls and ramps to full clock.
"""

import sys

sys.path.insert(0, "/opt/trn_rl_repo")

import numpy as np

import concourse.bass as bass
import concourse.tile as tile
from concourse import bacc, mybir
from concourse.bass_utils import run_bass_kernel_spmd
from concourse.masks import make_identity

F32 = mybir.dt.float32
F16 = mybir.dt.float16
ALU = mybir.AluOpType
ACTF = mybir.ActivationFunctionType
AXL = mybir.AxisListType

LN_EPS = 1e-5
NCORES = 8
SPB = 128          # segments (graphs) per block
T = 128            # nodes per tile
GRP = 4            # tiles per elementwise super-group
SB = 8             # tiles per stats batch (= 2 groups)


def _center(w, g):
    return ((w - w.mean(axis=1, keepdims=True)) * g[None, :]).astype(np.float32)


def _prep_host(inputs):
    x = np.asarray(inputs["x"], np.float32)
    y = np.asarray(inputs["y"], np.float32)
    batch = np.asarray(inputs["batch"], np.int64)
    N, E = x.shape
    B, YD = y.shape
    H = inputs["l0_lw1"].shape[1]

    for k in ("l0_lb1", "l0_lbt", "l0_lb2", "l0_gb1", "l0_gbt", "l0_gb2",
              "lr_lb1", "lr_lbt", "lr_lb2", "lr_gb1", "lr_gbt", "lr_gb2", "cb"):
        assert np.abs(np.asarray(inputs[k])).max() < 1e-12, f"{k} must be zero"
    for k in ("l0_lg", "l0_gg", "lr_lg", "lr_gg"):
        assert np.abs(np.asarray(inputs[k]) - 1.0).max() < 1e-12, f"{k} must be one"

    B_LOC = B // NCORES
    NBLK = B_LOC // SPB + 1     # one extra block frees seg-count slack
    seg_sizes = np.bincount(batch, minlength=B).astype(np.int64)
    seg_start = np.concatenate([[0], np.cumsum(seg_sizes)]).astype(np.int64)

    # Greedy-pack each core's 2048 segments into NBLK blocks of <=SPB
    # segments and <=TPB tiles; pick the smallest feasible uniform TPB.
    def pack(TPB):
        ranges = []          # ranges[c][k] = (seg_lo, seg_hi)
        for c in range(NCORES):
            s0, s1 = c * B_LOC, (c + 1) * B_LOC
            rr, s = [], s0
            for k in range(NBLK):
                rem_blocks = NBLK - 1 - k
                hi = min(s + SPB, s1)
                # cap nodes at TPB tiles
                while hi > s and seg_sizes[s:hi].sum() > TPB * T:
                    hi -= 1
                # must leave no more segments than remaining blocks can hold
                lo_req = s1 - rem_blocks * SPB
                if hi < lo_req:
                    return None
                rr.append((int(s), int(hi)))
                s = hi
            if s != s1:
                return None
            ranges.append(rr)
        return ranges

    for TPB in range(27, 34):
        ranges = pack(TPB)
        if ranges is not None:
            break
    assert ranges is not None
    MAXBLK = TPB
    NT = NBLK * MAXBLK          # tiles per core
    NPADC = NT * T              # padded nodes per core

    xT = [np.zeros((E, NPADC), np.float16) for _ in range(NCORES)]
    OT = [np.zeros((NBLK, T, MAXBLK, SPB), np.float16) for _ in range(NCORES)]
    OG = [np.zeros((NBLK, SPB, MAXBLK, T), np.float16) for _ in range(NCORES)]
    ysT = [np.zeros((YD, NBLK * SPB), np.float16) for _ in range(NCORES)]
    for c in range(NCORES):
        for k in range(NBLK):
            slo, shi = ranges[c][k]
            n0, n1 = int(seg_start[slo]), int(seg_start[shi])
            cnt = n1 - n0
            ysT[c][:, k * SPB:k * SPB + (shi - slo)] = y[slo:shi].T.astype(np.float16)
            if cnt == 0:
                continue
            base = k * MAXBLK * T
            xT[c][:, base:base + cnt] = x[n0:n1].T.astype(np.float16)
            a = np.arange(cnt)
            t = a // T
            p = a % T
            g = (batch[n0:n1] - slo).astype(np.int64)
            OT[c][k, p, t, g] = 1.0
            OG[c][k, g, t, p] = 1.0

    f16 = lambda w: np.ascontiguousarray(w).astype(np.float16)
    l0_w1f = _center(np.asarray(inputs["l0_lw1"], np.float32), np.asarray(inputs["l0_lg"], np.float32))
    W1X, W1G = [f16(l0_w1f)], [None]
    W2 = [f16(np.asarray(inputs["l0_lw2"], np.float32))]
    GW1 = [f16(_center(np.asarray(inputs["l0_gw1"], np.float32), np.asarray(inputs["l0_gg"], np.float32)))]
    GW2 = [f16(np.asarray(inputs["l0_gw2"], np.float32))]
    for i in range(2):
        w1f = _center(np.asarray(inputs["lr_lw1"][i], np.float32), np.asarray(inputs["lr_lg"][i], np.float32))
        W1X.append(f16(w1f[:E]))
        W1G.append(f16(w1f[E:]))
        W2.append(f16(np.asarray(inputs["lr_lw2"][i], np.float32)))
        GW1.append(f16(_center(np.asarray(inputs["lr_gw1"][i], np.float32), np.asarray(inputs["lr_gg"][i], np.float32))))
        GW2.append(f16(np.asarray(inputs["lr_gw2"][i], np.float32)))
    CW = f16(np.asarray(inputs["cw"], np.float32))

    geom = dict(N=N, E=E, B=B, YD=YD, H=H, B_LOC=NBLK * SPB, NBLK=NBLK,
                MAXBLK=MAXBLK, NT=NT, NPADC=NPADC, ranges=ranges)
    shared = dict(CW=CW)
    for l in range(3):
        shared[f"W1X{l}"] = W1X[l]
        shared[f"W2_{l}"] = W2[l]
        shared[f"GW1_{l}"] = GW1[l]
        shared[f"GW2_{l}"] = GW2[l]
        if l > 0:
            shared[f"W1G{l}"] = W1G[l]
    percore = [dict(xT=xT[c], OT=OT[c], OG=OG[c], ysT=ysT[c]) for c in range(NCORES)]
    return geom, shared, percore


def _build_program(geom):
    E, H, YD = geom["E"], geom["H"], geom["YD"]
    B_LOC, NBLK, MAXBLK, NT, NPADC = (geom["B_LOC"], geom["NBLK"],
                                      geom["MAXBLK"], geom["NT"], geom["NPADC"])
    HC = H // 128  # H chunks of 128
    NSB = (MAXBLK + SB - 1) // SB

    nc = bacc.Bacc("TRN2", target_bir_lowering=False, debug=False)

    xT_d = nc.dram_tensor("xT", [E, NPADC], F16, kind="ExternalInput").ap()
    OT_d = nc.dram_tensor("OT", [NBLK, T, MAXBLK, SPB], F16, kind="ExternalInput").ap()
    OG_d = nc.dram_tensor("OG", [NBLK, SPB, MAXBLK, T], F16, kind="ExternalInput").ap()
    ysT_d = nc.dram_tensor("ysT", [YD, B_LOC], F16, kind="ExternalInput").ap()
    CW_d = nc.dram_tensor("CW", [YD, E], F16, kind="ExternalInput").ap()
    W1X_d, W1G_d, W2_d, GW1_d, GW2_d = {}, {}, {}, {}, {}
    for l in range(3):
        W1X_d[l] = nc.dram_tensor(f"W1X{l}", [E, H], F16, kind="ExternalInput").ap()
        W2_d[l] = nc.dram_tensor(f"W2_{l}", [H, E], F16, kind="ExternalInput").ap()
        GW1_d[l] = nc.dram_tensor(f"GW1_{l}", [E, H], F16, kind="ExternalInput").ap()
        GW2_d[l] = nc.dram_tensor(f"GW2_{l}", [H, E], F16, kind="ExternalInput").ap()
        if l > 0:
            W1G_d[l] = nc.dram_tensor(f"W1G{l}", [E, H], F16, kind="ExternalInput").ap()
    outT_d = nc.dram_tensor("outT", [E, B_LOC], F32, kind="ExternalOutput").ap()

    with tile.TileContext(nc) as tc:
        with tc.tile_pool(name="const", bufs=1) as cpool, \
             tc.tile_pool(name="xin", bufs=3) as xpool, \
             tc.tile_pool(name="otin", bufs=3) as otpool, \
             tc.tile_pool(name="ogin", bufs=3) as ogpool, \
             tc.tile_pool(name="rstat", bufs=10) as spool, \
             tc.tile_pool(name="otr", bufs=20) as rpool, \
             tc.tile_pool(name="sqs", bufs=3) as sqpool, \
             tc.tile_pool(name="af", bufs=6) as afpool, \
             tc.tile_pool(name="segsb", bufs=3) as segsb, \
             tc.tile_pool(name="a1ps", bufs=2, space="PSUM") as a1pool, \
             tc.tile_pool(name="zps", bufs=2, space="PSUM") as zpool, \
             tc.tile_pool(name="segps", bufs=2, space="PSUM") as segps:

            # ---- resident constants ----
            def load_const(name, dram_ap, shape, rearr=None):
                tl = cpool.tile(shape, F16, tag=name)
                src = dram_ap if rearr is None else dram_ap.rearrange(rearr, c=HC)
                nc.sync.dma_start(tl[:], src)
                return tl

            w1x = {l: load_const(f"w1x{l}", W1X_d[l], [E, H]) for l in range(3)}
            w1g = {l: load_const(f"w1g{l}", W1G_d[l], [E, H]) for l in (1, 2)}
            gw1 = {l: load_const(f"gw1{l}", GW1_d[l], [E, H]) for l in range(3)}
            # w2 / gw2 as [128, HC, E] chunked stationary operands
            w2 = {l: load_const(f"w2{l}", W2_d[l], [128, HC, E], "(c p) e -> p c e")
                  for l in range(3)}
            gw2 = {l: load_const(f"gw2{l}", GW2_d[l], [128, HC, E], "(c p) e -> p c e")
                   for l in range(3)}
            cw = load_const("cw", CW_d, [YD, E])
            ys = load_const("ys", ysT_d, [YD, B_LOC])
            ident = cpool.tile([128, 128], F16, tag="ident")
            make_identity(nc, ident[:])
            eps_c = cpool.tile([128, 1], F32, tag="eps_c")
            nc.gpsimd.memset(eps_c[:], LN_EPS)
            xgw_store = cpool.tile([128, NBLK, H], F16, tag="xgw")

            gcount = [0]

            def back_batch(sbi, bsz, ss_b, groups, ot, z):
                """Stats + one-hot scaling + segment-sum matmuls for a
                completed stats batch (emitted one batch late)."""
                sd = spool.tile([T, bsz], F32, tag="sd")
                nc.scalar.activation(sd[:], ss_b[:], ACTF.Sqrt,
                                     bias=eps_c[:], scale=1.0 / H)
                r4 = spool.tile([T, bsz], F32, tag="r4")
                nc.vector.reciprocal(r4[:], sd[:])
                for goff, gsz, af in groups:
                    for j in range(gsz):
                        g = goff + j
                        ti = sbi * SB + g
                        otr = rpool.tile([T, SPB], F16, tag="otr")
                        nc.vector.tensor_scalar(
                            otr[:], ot[:, ti, :], r4[:, g:g + 1], 0.0,
                            ALU.mult, ALU.max)
                        nc.tensor.matmul(z[:], otr[:], af[:, j, :],
                                         start=(sbi == 0 and g == 0),
                                         stop=(sbi == NSB - 1 and g == bsz - 1))

            def seg_phase(l, blk, z):
                """Per-block segment pipeline: z -> s -> global MLP -> xg.
                Generator: yields between stages so the driver can interleave
                them with the next block's batches (keeps the PE queue free
                of head-of-line waits on the serial cross-engine chain)."""
                z_sb = segsb.tile([SPB, H], F16, tag="z_sb")
                nc.scalar.copy(z_sb[:], z[:])
                zT = segps.tile([128, HC, SPB], F16, tag="segps")
                for c in range(HC):
                    nc.tensor.transpose(zT[:, c, :], z_sb[:, c * 128:(c + 1) * 128], ident[:])
                zT_sb = segsb.tile([128, HC, SPB], F16, tag="zT_sb")
                nc.vector.tensor_copy(zT_sb[:], zT[:])
                yield
                sT = segps.tile([E, SPB], F32, tag="segps")
                for c in range(HC):
                    nc.tensor.matmul(sT[:], w2[l][:, c, :], zT_sb[:, c, :],
                                     start=(c == 0), stop=(c == HC - 1))
                sT_sb = segsb.tile([E, SPB], F16, tag="sT_sb")
                nc.scalar.copy(sT_sb[:], sT[:])
                yield
                ug = segps.tile([SPB, H], F32, tag="segps")
                nc.tensor.matmul(ug[:], sT_sb[:], gw1[l][:], start=True, stop=True)
                ssg = spool.tile([SPB, 1], F32, tag="ssg")
                sqg = segsb.tile([SPB, H], F16, tag="sqg")
                nc.scalar.activation(sqg[:], ug[:], ACTF.Square,
                                     accum_out=ssg[:])
                sdg = spool.tile([SPB, 1], F32, tag="sdg")
                nc.scalar.activation(sdg[:], ssg[:], ACTF.Sqrt,
                                     bias=eps_c[:], scale=1.0 / H)
                rg = spool.tile([SPB, 1], F32, tag="rg")
                nc.vector.reciprocal(rg[:], sdg[:])
                Rg = segsb.tile([SPB, H], F16, tag="Rg")
                nc.vector.tensor_scalar(
                    Rg[:], ug[:], rg[:], 0.0, ALU.mult, ALU.max)
                yield
                RgT = segps.tile([128, HC, SPB], F16, tag="segps")
                for c in range(HC):
                    nc.tensor.transpose(RgT[:, c, :], Rg[:, c * 128:(c + 1) * 128], ident[:])
                RgT_sb = segsb.tile([128, HC, SPB], F16, tag="RgT_sb")
                nc.vector.tensor_copy(RgT_sb[:], RgT[:])
                yield
                xgT = segps.tile([E, SPB], F32, tag="segps")
                for c in range(HC):
                    nc.tensor.matmul(xgT[:], gw2[l][:, c, :], RgT_sb[:, c, :],
                                     start=(c == 0),
                                     stop=(c == HC - 1 and l > 0))
                if l == 0:
                    nc.tensor.matmul(xgT[:], cw[:], ys[:, blk * SPB:(blk + 1) * SPB],
                                     start=False, stop=True)
                if l < 2:
                    xgT_sb = segsb.tile([E, SPB], F16, tag="xgT_sb")
                    nc.vector.tensor_copy(xgT_sb[:], xgT[:])
                    yield
                    xgw = segps.tile([SPB, H], F32, tag="segps")
                    nc.tensor.matmul(xgw[:], xgT_sb[:], w1g[l + 1][:],
                                     start=True, stop=True)
                    nc.scalar.copy(xgw_store[:, blk, :], xgw[:])
                else:
                    o_sb = segsb.tile([E, SPB], F32, tag="o_sb")
                    nc.vector.tensor_copy(o_sb[:], xgT[:])
                    nc.sync.dma_start(outT_d[:, blk * SPB:(blk + 1) * SPB], o_sb[:])

            def advance(gen):
                if gen is None:
                    return None
                try:
                    next(gen)
                    return gen
                except StopIteration:
                    return None

            pend_seg = None
            for l in range(3):
                for blk in range(NBLK):
                    xt = xpool.tile([E, MAXBLK * T], F16, tag="xt")
                    nc.sync.dma_start(xt[:], xT_d[:, blk * MAXBLK * T:(blk + 1) * MAXBLK * T])
                    ot = otpool.tile([T, MAXBLK, SPB], F16, tag="ot")
                    nc.sync.dma_start(ot[:], OT_d[blk])
                    if l > 0:
                        og = ogpool.tile([SPB, MAXBLK, T], F16, tag="og")
                        nc.sync.dma_start(og[:], OG_d[blk])
                    z = zpool.tile([SPB, H], F32, tag="z")
                    pend = None
                    for sbi in range(NSB):
                        bsz = min(SB, MAXBLK - sbi * SB)
                        ss_b = spool.tile([T, bsz], F16, tag="ssb")
                        sq_b = sqpool.tile([T, bsz, H], F16, tag="sq")
                        groups = []
                        for goff in range(0, bsz, GRP):
                            gsz = min(GRP, bsz - goff)
                            a1g = a1pool.tile([T, gsz, H], F32, tag="a1g")
                            for j in range(gsz):
                                ti = sbi * SB + goff + j
                                nc.tensor.matmul(a1g[:, j, :],
                                                 xt[:, ti * T:(ti + 1) * T],
                                                 w1x[l][:], start=True,
                                                 stop=(l == 0))
                                if l > 0:
                                    nc.tensor.matmul(a1g[:, j, :], og[:, ti, :],
                                                     xgw_store[:, blk, :],
                                                     start=False, stop=True)
                            nc.scalar.activation(
                                sq_b[:, goff:goff + gsz, :], a1g[:],
                                ACTF.Square)
                            af = afpool.tile([T, gsz, H], F16, tag="af")
                            if gcount[0] % 5 < 4:
                                nc.scalar.activation(af[:], a1g[:], ACTF.Relu)
                            else:
                                nc.vector.tensor_scalar(
                                    af[:], a1g[:], 1.0, 0.0, ALU.mult, ALU.max)
                            gcount[0] += 1
                            groups.append((goff, gsz, af))
                        with nc.allow_low_precision("LN stats tolerate f16"):
                            nc.vector.tensor_reduce(ss_b[:], sq_b[:],
                                                    AXL.X, ALU.add)
                        this = (sbi, bsz, ss_b, groups)
                        if pend is not None:
                            back_batch(*pend, ot, z)
                        pend_seg = advance(pend_seg)
                        pend = this
                    back_batch(*pend, ot, z)
                    # finish the previous block's segment stages before its z
                    # PSUM slot can be recycled (zpool bufs=2), then hand off
                    while pend_seg is not None:
                        pend_seg = advance(pend_seg)
                    pend_seg = seg_phase(l, blk, z)
            while pend_seg is not None:
                pend_seg = advance(pend_seg)

    nc.compile()
    return nc


def _run(inputs, trace=False):
    geom, shared, percore = _prep_host(inputs)
    nc = _build_program(geom)
    in_maps = []
    for c in range(NCORES):
        m = dict(shared)
        m.update(percore[c])
        in_maps.append(m)
    res = run_bass_kernel_spmd(nc, in_maps, list(range(NCORES)), trace=trace)
    B, E = geom["B"], geom["E"]
    out = np.empty((B, E), np.float32)
    for c in range(NCORES):
        oc = res.results[c]["outT"]
        for k, (slo, shi) in enumerate(geom["ranges"][c]):
            out[slo:shi] = oc[:, k * SPB:k * SPB + (shi - slo)].T
    return out, res


def kernel(**inputs):
    out, _ = _run(inputs)
    return out
